# revision 1
# baseline (speedup 1.0000x reference)
"""Trainium2 Bass kernel for the CurrentLIF recurrent spiking network.

Strategy: column-shard the 4096x4096 recurrent weight matrix across 8
NeuronCores (512 postsynaptic neurons each) so the (1-beta)-prescaled
weights stay SBUF-resident for all 500 steps. Each step: 32 chained
fp32 matmuls (binary spike vector stationary, W streaming) accumulate
the per-synapse-type drive in PSUM; the drive is transposed to a
neuron-major layout via the PE transpose path; the LIF state update
(two synaptic currents, membrane, refractory) runs on DVE in [128, 64]
tiles; the new spikes are masked/signed per presynaptic cell type,
cast to bf16 and exchanged with an 8-core AllGather; a single SWDGE
cast-DMA stages the gathered spikes as the next step's stationary
operand. The feed-forward input drive is precomputed on the host
(input spikes are known for all steps) and streamed per step.
"""
import numpy as np

import concourse.bass as bass
import concourse.bacc as bacc
import concourse.tile as tile
import concourse.mybir as mybir
import concourse.bass_utils as bass_utils

F32 = mybir.dt.float32
BF16 = mybir.dt.bfloat16
AL = mybir.AluOpType

DT = 1.0
BETA = float(np.float32(np.exp(-DT / 20.0)))
ALPHA0 = float(np.float32(np.exp(-DT / 5.0)))
ALPHA1 = float(np.float32(np.exp(-DT / 10.0)))
B = 16
NIN = 1024
N = 4096
NSH = 512
KC = 32

_CACHE = {}


def _build(T):
    nc = bacc.Bacc("TRN2", target_bir_lowering=False, debug=False, num_devices=8)
    Wt_d = nc.dram_tensor("wt", [128, KC * NSH], F32, kind="ExternalInput")
    FF_d = nc.dram_tensor("ff", [T, 128, 64], F32, kind="ExternalInput")
    M0_d = nc.dram_tensor("m0", [128, 64], F32, kind="ExternalInput")
    M1_d = nc.dram_tensor("m1", [128, 64], F32, kind="ExternalInput")
    ID_d = nc.dram_tensor("ident", [32, 32], F32, kind="ExternalInput")
    out_d = nc.dram_tensor("out", [T, 128, 64], F32, kind="ExternalOutput")

    with tile.TileContext(nc) as tc:
        with tc.tile_pool(name="big", bufs=1) as big, \
             tc.tile_pool(name="state", bufs=1) as state, \
             tc.tile_pool(name="work", bufs=3) as work, \
             tc.tile_pool(name="ffp", bufs=4) as ffp, \
             tc.tile_pool(name="psA", bufs=2, space="PSUM") as psA, \
             tc.tile_pool(name="psB", bufs=2, space="PSUM") as psB, \
             tc.tile_pool(name="dram", bufs=1, space="DRAM") as dram:

            Wt = big.tile([128, KC * NSH], F32, name="Wt")
            G = [big.tile([128, KC * 32], F32, name=f"G{i}", tag=f"G{i}")
                 for i in range(2)]
            nc.sync.dma_start(Wt[:], Wt_d[:])

            M0 = state.tile([128, 64], F32, name="M0")
            M1 = state.tile([128, 64], F32, name="M1")
            ident = state.tile([32, 32], F32, name="ident")
            nc.sync.dma_start(M0[:], M0_d[:])
            nc.sync.dma_start(M1[:], M1_d[:])
            nc.sync.dma_start(ident[:], ID_d[:])

            J0 = state.tile([128, 64], F32, name="J0")
            J1 = state.tile([128, 64], F32, name="J1")
            v = state.tile([128, 64], F32, name="v")
            refr = state.tile([128, 64], F32, name="refr")
            am = state.tile([128, 64], F32, name="am")
            for t_ in (J0, J1, v, refr):
                nc.gpsimd.memset(t_[:], 0.0)
            nc.gpsimd.memset(am[:], 1.0)
            for g_ in G:
                nc.gpsimd.memset(g_[:], 0.0)

            ag_in = [dram.tile([128, 4, 32], BF16, name=f"agin{i}", tag=f"agin{i}")
                     for i in range(2)]
            ag_out = [dram.tile([8, 128, 4, 32], BF16, name=f"agout{i}", tag=f"agout{i}")
                      for i in range(2)]

            PF = 3
            ff_tiles = {}
            for tpre in range(min(PF, T)):
                ft = ffp.tile([128, 64], F32, name=f"ff{tpre}", tag="ff")
                nc.sync.dma_start(ft[:], FF_d[:][tpre])
                ff_tiles[tpre] = ft

            for t in range(T):
                par = t % 2
                g = G[par]
                ff = ff_tiles.pop(t)

                nc.vector.tensor_scalar(J0[:], J0[:], ALPHA0, None, AL.mult)
                nc.vector.tensor_tensor(J0[:], J0[:], ff[:], AL.add)
                nc.vector.tensor_scalar(J1[:], J1[:], ALPHA1, None, AL.mult)
                nc.vector.tensor_scalar(v[:], v[:], BETA, None, AL.mult)

                dr = psA.tile([32, NSH], F32, name=f"dr{par}", tag=f"dr{par}")
                for k in range(KC):
                    nc.tensor.matmul(dr[:], g[:, 32 * k:32 * k + 32],
                                     Wt[:, NSH * k:NSH * k + NSH],
                                     start=(k == 0), stop=(k == KC - 1))
                drc = work.tile([32, NSH], F32, name="drc", tag="drc")
                nc.scalar.copy(drc[:], dr[:])
                tp = psB.tile([128, 128], F32, name=f"tp{par}", tag=f"tp{par}")
                for q in range(4):
                    nc.tensor.transpose(tp[:, 32 * q:32 * q + 32],
                                        drc[:, 128 * q:128 * q + 128], ident[:])
                tpr = tp[:].rearrange("p (q s b) -> p q s b", q=4, s=2)
                J0r = J0[:].rearrange("p (q b) -> p q b", q=4)
                J1r = J1[:].rearrange("p (q b) -> p q b", q=4)
                nc.vector.tensor_tensor(J0r, J0r, tpr[:, :, 0, :], AL.add)
                nc.vector.tensor_tensor(J1r, J1r, tpr[:, :, 1, :], AL.add)

                it = work.tile([128, 64], F32, name="it", tag="it")
                nc.vector.tensor_tensor(it[:], J0[:], J1[:], AL.add)
                nc.vector.tensor_tensor(v[:], v[:], it[:], AL.add)
                nc.vector.tensor_tensor(v[:], v[:], am[:], AL.mult)
                s = work.tile([128, 64], F32, name="s", tag="s")
                nc.vector.tensor_scalar(s[:], v[:], 1.0, None, AL.is_gt)

                nc.scalar.dma_start(out_d[:][t], s[:])

                if t < T - 1:
                    stg = work.tile([128, 128], BF16, name="stg", tag="stg")
                    sr = stg[:].rearrange("p (q s b) -> p q s b", q=4, s=2)
                    srs = s[:].rearrange("p (q b) -> p q b", q=4)
                    nc.vector.tensor_tensor(
                        sr[:, :, 0, :], srs,
                        M0[:].rearrange("p (q b) -> p q b", q=4), AL.mult)
                    nc.vector.tensor_tensor(
                        sr[:, :, 1, :], srs,
                        M1[:].rearrange("p (q b) -> p q b", q=4), AL.mult)
                    nc.sync.dma_start(ag_in[par][:],
                                      stg[:].rearrange("p (q c) -> p q c", q=4))
                    nc.gpsimd.collective_compute(
                        "AllGather", AL.bypass, replica_groups=[list(range(8))],
                        ins=[ag_in[par].opt()], outs=[ag_out[par].opt()])
                    gn = G[1 - par]
                    nc.gpsimd.dma_start(
                        gn[:].rearrange("p (r x) -> p r x", r=8),
                        ag_out[par][:].rearrange("r p q c -> p r (q c)"))

                ns = work.tile([128, 64], F32, name="ns", tag="ns")
                nc.vector.tensor_scalar(ns[:], s[:], -1.0, 1.0, AL.mult, AL.add)
                nc.vector.tensor_tensor(v[:], v[:], ns[:], AL.mult)
                nc.vector.tensor_scalar(refr[:], refr[:], -0.5, 0.0, AL.add, AL.max)
                nc.vector.tensor_tensor(refr[:], refr[:], s[:], AL.add)
                nc.vector.tensor_scalar(am[:], refr[:], 0.0, None, AL.is_le)

                if t + PF < T:
                    ft = ffp.tile([128, 64], F32, name=f"ff{t+PF}", tag="ff")
                    nc.sync.dma_start(ft[:], FF_d[:][t + PF])
                    ff_tiles[t + PF] = ft
    nc.compile()
    return nc


def _prep_inputs(input_spikes, W, W_FF, cell_type_indices, T):
    beta = np.float32(BETA)
    Wp = ((np.float32(1.0) - beta) * W.astype(np.float32)).astype(np.float32)
    WFFp = ((np.float32(1.0) - beta) * W_FF.astype(np.float32)).astype(np.float32)
    B_, Tf, NIN_ = input_spikes.shape
    sp = input_spikes.astype(np.float32).transpose(1, 0, 2).reshape(Tf * B_, NIN_)[:T * B_]
    ff_all = (sp @ WFFp).reshape(T, B_, N)
    cti = np.asarray(cell_type_indices).astype(np.int32)
    ident = np.eye(32, dtype=np.float32)
    in_maps = []
    for c in range(8):
        Wc = Wp[:, 512 * c:512 * (c + 1)]
        Wt = Wc.reshape(32, 128, 512).transpose(1, 0, 2).reshape(128, 32 * 512).copy()
        ffc = ff_all[:, :, 512 * c:512 * (c + 1)]
        FF = ffc.reshape(T, B_, 4, 128).transpose(0, 3, 2, 1).reshape(T, 128, 64).copy()
        ctic = cti[512 * c:512 * (c + 1)].reshape(4, 128)
        m0 = (ctic == 0).astype(np.float32)
        m1 = -(ctic == 1).astype(np.float32)
        M0 = np.repeat(m0.T[:, :, None], 16, axis=2).reshape(128, 64).copy()
        M1 = np.repeat(m1.T[:, :, None], 16, axis=2).reshape(128, 64).copy()
        in_maps.append({"wt": Wt, "ff": FF, "m0": M0, "m1": M1, "ident": ident})
    return in_maps


def _assemble(results, T):
    cols = []
    for c in range(8):
        arr = results[c]["out"].reshape(T, 128, 4, 16)
        cols.append(arr.transpose(3, 0, 2, 1).reshape(B, T, 512))
    return np.concatenate(cols, axis=2).astype(np.float32)


def kernel(input_spikes, W, W_FF, cell_type_indices):
    T = int(input_spikes.shape[1])
    if T not in _CACHE:
        _CACHE[T] = _build(T)
    nc = _CACHE[T]
    in_maps = _prep_inputs(np.asarray(input_spikes), np.asarray(W),
                           np.asarray(W_FF), np.asarray(cell_type_indices), T)
    res = bass_utils.run_bass_kernel_spmd(nc, in_maps, core_ids=list(range(8)))
    return _assemble(res.results, T)

